# revision 7
# baseline (speedup 1.0000x reference)
"""MoCo grouped-queue logits kernel for Trainium2 (8 NeuronCores, Bass/Tile).

Computation (reference):
    q = l2norm(im_q @ W_q)          # [N, C]
    k = l2norm(im_k @ W_k)          # [N, C]
    l_pos[n] = q[n] . k[n]
    route[n] = (label[n] - 1) % 4
    l_neg[n, :] = q[n] @ queues[route[n]]    # [N, K]
    logits = concat([l_pos, l_neg], 1) / T   # [N, 1+K]
    labels = zeros(N)

Strategy:
  - Queues are sharded along K across the 8 cores ([4, 128, K/8] each);
    each core computes all N samples against its K-slice. Each queue
    byte is read exactly once chip-wide.
  - Samples are SORTED by route group on the host, so l_neg becomes a
    few dense [cnt<=128, 512]-tile matmuls, one group per tile — no
    masking and no 4x redundant PE work. The tile plan depends only on
    the per-group histogram; compiled programs are cached per plan.
  - The q-projection (needed by every core) is replicated; the l_pos
    path (q.k) only needs per-sample values, so its projections are
    sharded: core i computes l_pos for sorted samples [64i, 64i+64).
  - Host work is layout only: transpose/tile inputs, sort rows,
    unsort output rows.
"""

import numpy as np

# Problem constants (hardcoded; kernel.py must be self-contained).
N = 512          # batch
D = 2048         # input feature dim
C = 128          # embedding dim
K = 65536        # queue length
G = 4            # number of queues
T = 0.07         # softmax temperature
NCORES = 8
KSH = K // NCORES            # 8192 queue columns per core
DT = D // 128                # 16 contraction tiles for the projections
NLP = N // NCORES            # 64 l_pos samples per core
CW = 2048                    # queue-chunk width (columns per DMA chunk)
NKC = KSH // CW              # 4 chunks per core
NSUB = CW // 512             # 4 matmuls (N=512) per chunk

_prog_cache = {}


def _plan_from_counts(counts):
    """M-tile plan: list of (row0, cnt<=128, group) over sorted rows.

    Groups with more than 128 rows are covered by full 128-row tiles whose
    last tile is [end-128, end) — tiles may OVERLAP (overlapped rows are
    computed and written twice with identical values). This keeps nearly
    every output DMA at full 128 partitions, which balances the SDMA
    engines (partial-partition DMAs concentrate on a few engines).
    """
    tiles = []
    r0 = 0
    for g in range(G):
        c = int(counts[g])
        if c == 0:
            continue
        if c <= 128:
            tiles.append((r0, c, g))
        else:
            off = 0
            while off + 128 < c:
                tiles.append((r0 + off, 128, g))
                off += 128
            tiles.append((r0 + c - 128, 128, g))
        r0 += c
    return tuple(tiles)


def _build(plan):
    """Build + compile the Bass program for one tile plan."""
    import concourse.tile as tile
    from concourse import bacc, mybir

    f32 = mybir.dt.float32
    AF = mybir.ActivationFunctionType

    nc = bacc.Bacc("TRN2", target_bir_lowering=False, debug=False,
                   num_devices=NCORES)

    # Inputs, pre-tiled on host so every DMA is partition-contiguous.
    #   imqt  [128, DT*512]: [p, t*512+n] = im_q_sorted[n, t*128+p]
    #   wqt   [128, DT*128]: [p, t*128+c] = W_q[t*128+p, c]  (same wkt)
    #   imqlp/imklp [128, DT*64]: this core's 64 sorted samples
    #   qsh   [G, 128, KSH]: this core's K-slice of the queues
    imqt = nc.dram_tensor("imqt", [128, DT * 512], f32, kind="ExternalInput")
    wqt = nc.dram_tensor("wqt", [128, DT * 128], f32, kind="ExternalInput")
    wkt = nc.dram_tensor("wkt", [128, DT * 128], f32, kind="ExternalInput")
    imqlp = nc.dram_tensor("imqlp", [128, DT * NLP], f32, kind="ExternalInput")
    imklp = nc.dram_tensor("imklp", [128, DT * NLP], f32, kind="ExternalInput")
    qsh = nc.dram_tensor("qsh", [G, 128, KSH], f32, kind="ExternalInput")
    # Outputs (sorted row order): lneg [N, KSH], lpos [1, NLP] (scaled 1/T).
    lneg = nc.dram_tensor("lneg", [N, KSH], f32, kind="ExternalOutput")
    lpos = nc.dram_tensor("lpos", [1, NLP], f32, kind="ExternalOutput")

    used_groups = sorted({g for _, _, g in plan})

    with tile.TileContext(nc) as tc:
        with tc.tile_pool(name="pers", bufs=1) as pers:
            ones_col = pers.tile([128, 1], f32)
            nc.vector.memset(ones_col[:], 1.0)
            ones_row = pers.tile([1, 128], f32)
            nc.vector.memset(ones_row[:], 1.0)
            # Sorted qT scaled by invnorm/T: the stationary operand.
            qts = pers.tile([128, N], f32)

            # ---- Phase A: projections, norms, l_pos shard ----
            with tc.tile_pool(name="pa", bufs=1) as pa, \
                 tc.tile_pool(name="paps", bufs=1, space="PSUM") as paps:
                # Order matters: the q-projection (wqt+imqt) gates phase A,
                # so those ride the HWDGE ring first.
                wq_sb = pa.tile([128, DT * 128], f32)
                nc.sync.dma_start(wq_sb[:], wqt[:])
                imq_sb = pa.tile([128, DT * 512], f32)
                nc.sync.dma_start(imq_sb[:], imqt[:])
                imqlp_sb = pa.tile([128, DT * NLP], f32)
                nc.sync.dma_start(imqlp_sb[:], imqlp[:])
                imklp_sb = pa.tile([128, DT * NLP], f32)
                nc.sync.dma_start(imklp_sb[:], imklp[:])
                wk_sb = pa.tile([128, DT * 128], f32)
                nc.sync.dma_start(wk_sb[:], wkt[:])

                # qT[c, n] = sum_d W_q[d, c] im_q[n, d]  (all N samples)
                qt_ps = paps.tile([128, N], f32)
                for t in range(DT):
                    nc.tensor.matmul(qt_ps[:],
                                     wq_sb[:, t * 128:(t + 1) * 128],
                                     imq_sb[:, t * 512:(t + 1) * 512],
                                     start=(t == 0), stop=(t == DT - 1))
                # l_pos shard projections (64 samples each for q and k).
                qlp_ps = paps.tile([128, NLP], f32)
                for t in range(DT):
                    nc.tensor.matmul(qlp_ps[:],
                                     wq_sb[:, t * 128:(t + 1) * 128],
                                     imqlp_sb[:, t * NLP:(t + 1) * NLP],
                                     start=(t == 0), stop=(t == DT - 1))
                klp_ps = paps.tile([128, NLP], f32)
                for t in range(DT):
                    nc.tensor.matmul(klp_ps[:],
                                     wk_sb[:, t * 128:(t + 1) * 128],
                                     imklp_sb[:, t * NLP:(t + 1) * NLP],
                                     start=(t == 0), stop=(t == DT - 1))

                qt_sb = pa.tile([128, N], f32)
                nc.vector.tensor_copy(qt_sb[:], qt_ps[:])
                qlp_sb = pa.tile([128, NLP], f32)
                nc.vector.tensor_copy(qlp_sb[:], qlp_ps[:])
                klp_sb = pa.tile([128, NLP], f32)
                nc.vector.tensor_copy(klp_sb[:], klp_ps[:])

                # Column sums over partitions via ones-vector matmuls.
                sqq = pa.tile([128, N], f32)
                nc.vector.tensor_mul(sqq[:], qt_sb[:], qt_sb[:])
                ssqq_ps = paps.tile([1, N], f32)
                nc.tensor.matmul(ssqq_ps[:], ones_col[:], sqq[:],
                                 start=True, stop=True)

                red_sb = pa.tile([128, 3 * NLP], f32)
                nc.vector.tensor_mul(red_sb[:, 0:NLP],
                                     qlp_sb[:], qlp_sb[:])
                nc.vector.tensor_mul(red_sb[:, NLP:2 * NLP],
                                     klp_sb[:], klp_sb[:])
                nc.vector.tensor_mul(red_sb[:, 2 * NLP:3 * NLP],
                                     qlp_sb[:], klp_sb[:])
                red_ps = paps.tile([1, 3 * NLP], f32)
                nc.tensor.matmul(red_ps[:], ones_col[:], red_sb[:],
                                 start=True, stop=True)

                # inv = 1 / max(sqrt(ssq), 1e-12), for q (all N).
                normq = pa.tile([1, N], f32)
                nc.scalar.activation(normq[:], ssqq_ps[:], AF.Sqrt)
                normqc = pa.tile([1, N], f32)
                nc.vector.tensor_scalar_max(normqc[:], normq[:], 1e-12)
                invq = pa.tile([1, N], f32)
                nc.vector.reciprocal(invq[:], normqc[:])
                invq_t = pa.tile([1, N], f32)
                nc.vector.tensor_scalar_mul(invq_t[:], invq[:], 1.0 / T)

                # l_pos shard: dot * invq_lp * invk_lp / T.
                norml = pa.tile([1, 2 * NLP], f32)
                nc.scalar.activation(norml[:], red_ps[:, 0:2 * NLP], AF.Sqrt)
                normlc = pa.tile([1, 2 * NLP], f32)
                nc.vector.tensor_scalar_max(normlc[:], norml[:], 1e-12)
                invl = pa.tile([1, 2 * NLP], f32)
                nc.vector.reciprocal(invl[:], normlc[:])
                lp1 = pa.tile([1, NLP], f32)
                nc.vector.tensor_mul(lp1[:], invl[:, 0:NLP], invl[:, NLP:2 * NLP])
                lp2 = pa.tile([1, NLP], f32)
                nc.vector.tensor_mul(lp2[:], lp1[:], red_ps[:, 2 * NLP:3 * NLP])
                lp3 = pa.tile([1, NLP], f32)
                nc.vector.tensor_scalar_mul(lp3[:], lp2[:], 1.0 / T)
                nc.sync.dma_start(lpos[:], lp3[:])

                # qts = qT * broadcast(invq/T): outer(ones, invq_t) via PE.
                invqb_ps = paps.tile([128, N], f32)
                nc.tensor.matmul(invqb_ps[:], ones_row[:], invq_t[:],
                                 start=True, stop=True)
                nc.vector.tensor_mul(qts[:], qt_sb[:], invqb_ps[:])

            # ---- Phase C: l_neg tiles over the queue shard ----
            with tc.tile_pool(name="qp", bufs=3) as qp, \
                 tc.tile_pool(name="sp", bufs=6) as sp, \
                 tc.tile_pool(name="cps", bufs=6, space="PSUM") as cps:
                for kc in range(NKC):
                    qch = {}
                    for g in used_groups:
                        qt_ = qp.tile([128, CW], f32, tag=f"qch{g}",
                                      name=f"qch{g}_{kc}")
                        nc.scalar.dma_start(
                            qt_[:], qsh[g, :, kc * CW:(kc + 1) * CW])
                        qch[g] = qt_
                    for ti, (r0, cnt, g) in enumerate(plan):
                        stg = sp.tile([128, CW], f32, tag="stg",
                                      name=f"stg_{kc}_{ti}")
                        for sub in range(NSUB):
                            ps = cps.tile([128, 512], f32, tag="ps",
                                          name=f"ps_{kc}_{ti}_{sub}")
                            nc.tensor.matmul(
                                ps[:cnt, :],
                                qts[:, r0:r0 + cnt],
                                qch[g][:, sub * 512:(sub + 1) * 512],
                                start=True, stop=True)
                            # Split PSUM->SBUF copies across DVE and ACT
                            # (~3:1 throughput) so neither serializes PE.
                            if sub == 0:
                                nc.scalar.copy(
                                    stg[:cnt, sub * 512:(sub + 1) * 512],
                                    ps[:cnt, :])
                            else:
                                nc.vector.tensor_copy(
                                    stg[:cnt, sub * 512:(sub + 1) * 512],
                                    ps[:cnt, :])
                        nc.sync.dma_start(
                            lneg[r0:r0 + cnt, kc * CW:(kc + 1) * CW],
                            stg[:cnt, :])

    nc.compile()
    return nc


def _get_program(plan):
    if plan not in _prog_cache:
        _prog_cache[plan] = _build(plan)
    return _prog_cache[plan]


def _tile_cols(x, ncols):
    """[n, D] -> [128, DT*n] with [p, t*n+j] = x[j, t*128+p]."""
    n = x.shape[0]
    assert n == ncols
    return np.ascontiguousarray(
        x.T.reshape(DT, 128, n).transpose(1, 0, 2).reshape(128, DT * n))


def _stage_inputs(im_q, im_k, W_q, W_k, queues, label):
    f32 = np.float32
    im_q = np.asarray(im_q, dtype=f32)
    im_k = np.asarray(im_k, dtype=f32)
    W_q = np.asarray(W_q, dtype=f32)
    W_k = np.asarray(W_k, dtype=f32)
    queues = np.asarray(queues, dtype=f32)
    label = np.asarray(label)

    route = ((label.astype(np.int64) - 1) % G).astype(np.int64)
    order = np.argsort(route, kind="stable")
    counts = np.bincount(route, minlength=G)
    plan = _plan_from_counts(counts)

    im_q_s = im_q[order]
    im_k_s = im_k[order]

    imqt = _tile_cols(im_q_s, N)
    wqt = np.ascontiguousarray(
        W_q.reshape(DT, 128, C).transpose(1, 0, 2).reshape(128, DT * C))
    wkt = np.ascontiguousarray(
        W_k.reshape(DT, 128, C).transpose(1, 0, 2).reshape(128, DT * C))

    in_maps = []
    for i in range(NCORES):
        sl = slice(i * NLP, (i + 1) * NLP)
        in_maps.append({
            "imqt": imqt, "wqt": wqt, "wkt": wkt,
            "imqlp": _tile_cols(im_q_s[sl], NLP),
            "imklp": _tile_cols(im_k_s[sl], NLP),
            "qsh": np.ascontiguousarray(queues[:, :, i * KSH:(i + 1) * KSH]),
        })
    return plan, order, in_maps


def kernel(im_q, im_k, W_q, W_k, queues, label):
    from concourse.bass_utils import run_bass_kernel_spmd

    plan, order, in_maps = _stage_inputs(im_q, im_k, W_q, W_k, queues, label)
    nc = _get_program(plan)
    res = run_bass_kernel_spmd(nc, in_maps, core_ids=list(range(NCORES)))

    logits = np.empty((N, 1 + K), dtype=np.float32)
    lpos_sorted = np.concatenate(
        [res.results[i]["lpos"][0] for i in range(NCORES)])
    logits[order, 0] = lpos_sorted
    for i in range(NCORES):
        logits[order, 1 + i * KSH:1 + (i + 1) * KSH] = res.results[i]["lneg"]
    labels = np.zeros(N, dtype=np.int32)
    return logits, labels


# revision 13
# speedup vs baseline: 1.0888x; 1.0888x over previous
"""MoCo grouped-queue logits kernel for Trainium2 (8 NeuronCores, Bass/Tile).

Computation (reference):
    q = l2norm(im_q @ W_q)          # [N, C]
    k = l2norm(im_k @ W_k)          # [N, C]
    l_pos[n] = q[n] . k[n]
    route[n] = (label[n] - 1) % 4
    l_neg[n, :] = q[n] @ queues[route[n]]    # [N, K]
    logits = concat([l_pos, l_neg], 1) / T   # [N, 1+K]
    labels = zeros(N)

Strategy:
  - Queues are sharded along K across the 8 cores ([4, 128, K/8] each);
    each core computes all N samples against its K-slice. Each queue
    byte is read exactly once chip-wide.
  - Samples are SORTED by route group on the host, so l_neg becomes a
    few dense [cnt<=128, 512]-tile matmuls, one group per tile — no
    masking and no 4x redundant PE work. The tile plan depends only on
    the per-group histogram; compiled programs are cached per plan.
  - The q-projection (needed by every core) is replicated; the l_pos
    path (q.k) only needs per-sample values, so its projections are
    sharded: core i computes l_pos for sorted samples [64i, 64i+64).
  - Host work is layout only: transpose/tile inputs, sort rows,
    unsort output rows.
"""

import numpy as np

# Problem constants (hardcoded; kernel.py must be self-contained).
N = 512          # batch
D = 2048         # input feature dim
C = 128          # embedding dim
K = 65536        # queue length
G = 4            # number of queues
T = 0.07         # softmax temperature
NCORES = 8
KSH = K // NCORES            # 8192 queue columns per core
DT = D // 128                # 16 contraction tiles for the projections
NLP = N // NCORES            # 64 l_pos samples per core
CW = 2048                    # queue-chunk width (columns per DMA chunk)
NKC = KSH // CW              # 4 chunks per core
NSUB = CW // 512             # 4 matmuls (N=512) per chunk

_prog_cache = {}


def _plan_from_counts(counts):
    """M-tile plan: list of (row0, cnt<=128, group) over sorted rows.

    Groups with more than 128 rows are covered by full 128-row tiles whose
    last tile is [end-128, end) — tiles may OVERLAP (overlapped rows are
    computed and written twice with identical values). This keeps nearly
    every output DMA at full 128 partitions, which balances the SDMA
    engines (partial-partition DMAs concentrate on a few engines).
    """
    tiles = []
    r0 = 0
    for g in range(G):
        c = int(counts[g])
        if c == 0:
            continue
        if c <= 128:
            tiles.append((r0, c, g))
        else:
            off = 0
            while off + 128 < c:
                tiles.append((r0 + off, 128, g))
                off += 128
            tiles.append((r0 + c - 128, 128, g))
        r0 += c
    return tuple(tiles)


def _build(plan):
    """Build + compile the Bass program for one tile plan."""
    import concourse.tile as tile
    from concourse import bacc, mybir

    f32 = mybir.dt.float32
    AF = mybir.ActivationFunctionType

    nc = bacc.Bacc("TRN2", target_bir_lowering=False, debug=False,
                   num_devices=NCORES)

    # Inputs, pre-tiled on host so every DMA is partition-contiguous.
    #   imqt  [128, DT*512]: [p, t*512+n] = im_q_sorted[n, t*128+p]
    #   wqt   [128, DT*128]: [p, t*128+c] = W_q[t*128+p, c]  (same wkt)
    #   imqlp/imklp [128, DT*64]: this core's 64 sorted samples
    #   qsh   [G, 128, KSH]: this core's K-slice of the queues
    imqt = nc.dram_tensor("imqt", [128, DT * 512], f32, kind="ExternalInput")
    wqt = nc.dram_tensor("wqt", [128, DT * 128], f32, kind="ExternalInput")
    wkt = nc.dram_tensor("wkt", [128, DT * 128], f32, kind="ExternalInput")
    imqlp = nc.dram_tensor("imqlp", [128, DT * NLP], f32, kind="ExternalInput")
    imklp = nc.dram_tensor("imklp", [128, DT * NLP], f32, kind="ExternalInput")
    qsh = nc.dram_tensor("qsh", [G, 128, KSH], f32, kind="ExternalInput")
    # Outputs (sorted row order): lneg [N, KSH], lpos [1, NLP] (scaled 1/T).
    lneg = nc.dram_tensor("lneg", [N, KSH], f32, kind="ExternalOutput")
    lpos = nc.dram_tensor("lpos", [1, NLP], f32, kind="ExternalOutput")

    used_groups = sorted({g for _, _, g in plan})

    with tile.TileContext(nc) as tc:
        with tc.tile_pool(name="pers", bufs=1) as pers:
            ones_col = pers.tile([128, 1], f32)
            nc.vector.memset(ones_col[:], 1.0)
            # 1/T folded into the broadcast outer-product constant.
            invt_row = pers.tile([1, 128], f32)
            nc.vector.memset(invt_row[:], 1.0 / T)
            eps_bias = pers.tile([1, 1], f32)
            nc.vector.memset(eps_bias[:], 1e-24)
            # Sorted qT scaled by invnorm/T: the stationary operand.
            qts = pers.tile([128, N], f32)

            # ---- Phase A: projections, norms, l_pos shard ----
            with tc.tile_pool(name="pa", bufs=1) as pa, \
                 tc.tile_pool(name="paps", bufs=1, space="PSUM") as paps:
                # Order matters: the q-projection (wqt+imqt) gates phase A,
                # so those ride the HWDGE ring first.
                wq_sb = pa.tile([128, DT * 128], f32)
                nc.sync.dma_start(wq_sb[:], wqt[:])
                imq_sb = pa.tile([128, DT * 512], f32)
                nc.sync.dma_start(imq_sb[:], imqt[:])
                imqlp_sb = pa.tile([128, DT * NLP], f32)
                nc.sync.dma_start(imqlp_sb[:], imqlp[:])
                imklp_sb = pa.tile([128, DT * NLP], f32)
                nc.sync.dma_start(imklp_sb[:], imklp[:])
                wk_sb = pa.tile([128, DT * 128], f32)
                nc.sync.dma_start(wk_sb[:], wkt[:])

                # qT[c, n] = sum_d W_q[d, c] im_q[n, d]  (all N samples)
                qt_ps = paps.tile([128, N], f32)
                for t in range(DT):
                    nc.tensor.matmul(qt_ps[:],
                                     wq_sb[:, t * 128:(t + 1) * 128],
                                     imq_sb[:, t * 512:(t + 1) * 512],
                                     start=(t == 0), stop=(t == DT - 1))
                # l_pos shard projections (64 samples each for q and k).
                qlp_ps = paps.tile([128, NLP], f32)
                for t in range(DT):
                    nc.tensor.matmul(qlp_ps[:],
                                     wq_sb[:, t * 128:(t + 1) * 128],
                                     imqlp_sb[:, t * NLP:(t + 1) * NLP],
                                     start=(t == 0), stop=(t == DT - 1))
                klp_ps = paps.tile([128, NLP], f32)
                for t in range(DT):
                    nc.tensor.matmul(klp_ps[:],
                                     wk_sb[:, t * 128:(t + 1) * 128],
                                     imklp_sb[:, t * NLP:(t + 1) * NLP],
                                     start=(t == 0), stop=(t == DT - 1))

                qt_sb = pa.tile([128, N], f32)
                nc.vector.tensor_copy(qt_sb[:], qt_ps[:])
                qlp_sb = pa.tile([128, NLP], f32)
                nc.vector.tensor_copy(qlp_sb[:], qlp_ps[:])
                klp_sb = pa.tile([128, NLP], f32)
                nc.vector.tensor_copy(klp_sb[:], klp_ps[:])

                # Column sums over partitions via ones-vector matmuls.
                sqq = pa.tile([128, N], f32)
                nc.vector.tensor_mul(sqq[:], qt_sb[:], qt_sb[:])
                ssqq_ps = paps.tile([1, N], f32)
                nc.tensor.matmul(ssqq_ps[:], ones_col[:], sqq[:],
                                 start=True, stop=True)

                red_sb = pa.tile([128, 3 * NLP], f32)
                nc.vector.tensor_mul(red_sb[:, 0:NLP],
                                     qlp_sb[:], qlp_sb[:])
                nc.vector.tensor_mul(red_sb[:, NLP:2 * NLP],
                                     klp_sb[:], klp_sb[:])
                nc.vector.tensor_mul(red_sb[:, 2 * NLP:3 * NLP],
                                     qlp_sb[:], klp_sb[:])
                red_ps = paps.tile([1, 3 * NLP], f32)
                nc.tensor.matmul(red_ps[:], ones_col[:], red_sb[:],
                                 start=True, stop=True)

                # inv = 1/max(sqrt(ssq), 1e-12) == 1/sqrt(ssq + 1e-24) up to
                # fp32 precision for any non-degenerate ssq; the bias folds
                # the clamp into the Sqrt activation.
                normq = pa.tile([1, N], f32)
                nc.scalar.activation(normq[:], ssqq_ps[:], AF.Sqrt,
                                     bias=eps_bias[:])
                invq = pa.tile([1, N], f32)
                nc.vector.reciprocal(invq[:], normq[:])

                # l_pos shard: dot * invq_lp * invk_lp / T.
                norml = pa.tile([1, 2 * NLP], f32)
                nc.scalar.activation(norml[:], red_ps[:, 0:2 * NLP], AF.Sqrt,
                                     bias=eps_bias[:])
                invl = pa.tile([1, 2 * NLP], f32)
                nc.vector.reciprocal(invl[:], norml[:])
                lp1 = pa.tile([1, NLP], f32)
                nc.vector.tensor_mul(lp1[:], invl[:, 0:NLP], invl[:, NLP:2 * NLP])
                lp2 = pa.tile([1, NLP], f32)
                nc.vector.tensor_mul(lp2[:], lp1[:], red_ps[:, 2 * NLP:3 * NLP])
                lp3 = pa.tile([1, NLP], f32)
                nc.vector.tensor_scalar_mul(lp3[:], lp2[:], 1.0 / T)
                nc.sync.dma_start(lpos[:], lp3[:])

                # qts = qT * broadcast(invq/T): outer(1/T, invq) via PE.
                invqb_ps = paps.tile([128, N], f32)
                nc.tensor.matmul(invqb_ps[:], invt_row[:], invq[:],
                                 start=True, stop=True)
                nc.vector.tensor_mul(qts[:], qt_sb[:], invqb_ps[:])

            # ---- Phase C: l_neg tiles over the queue shard ----
            # fresh0[ti]: first row of tile ti not already written by the
            # previous (overlapping) tile — only fresh rows go to HBM.
            fresh0 = []
            prev_end = 0
            for r0, cnt, g in plan:
                fresh0.append(max(r0, prev_end))
                prev_end = max(prev_end, r0 + cnt)

            with tc.tile_pool(name="qp", bufs=3) as qp, \
                 tc.tile_pool(name="sp", bufs=4) as sp, \
                 tc.tile_pool(name="cps", bufs=2, space="PSUM") as cps:
                for kc in range(NKC):
                    qch = {}
                    for g in used_groups:
                        qt_ = qp.tile([128, CW], f32, tag=f"qch{g}",
                                      name=f"qch{g}_{kc}")
                        nc.scalar.dma_start(
                            qt_[:], qsh[g, :, kc * CW:(kc + 1) * CW])
                        qch[g] = qt_
                    for ti, (r0, cnt, g) in enumerate(plan):
                        # One 4-bank PSUM tile per (tile, chunk); the four
                        # N=512 matmuls land in separate banks, then ONE
                        # DVE copy drains all of them (fewer sem chains).
                        ps = cps.tile([128, CW], f32, tag="ps",
                                      name=f"ps_{kc}_{ti}")
                        for sub in range(NSUB):
                            nc.tensor.matmul(
                                ps[:cnt, sub * 512:(sub + 1) * 512],
                                qts[:, r0:r0 + cnt],
                                qch[g][:, sub * 512:(sub + 1) * 512],
                                start=True, stop=True)
                        stg = sp.tile([128, CW], f32, tag="stg",
                                      name=f"stg_{kc}_{ti}")
                        nc.vector.tensor_copy(stg[:cnt, :], ps[:cnt, :])
                        f0 = fresh0[ti] - r0
                        nc.sync.dma_start(
                            lneg[fresh0[ti]:r0 + cnt, kc * CW:(kc + 1) * CW],
                            stg[f0:cnt, :])

    nc.compile()
    return nc


def _get_program(plan):
    if plan not in _prog_cache:
        _prog_cache[plan] = _build(plan)
    return _prog_cache[plan]


def _tile_cols(x, ncols):
    """[n, D] -> [128, DT*n] with [p, t*n+j] = x[j, t*128+p]."""
    n = x.shape[0]
    assert n == ncols
    return np.ascontiguousarray(
        x.T.reshape(DT, 128, n).transpose(1, 0, 2).reshape(128, DT * n))


def _stage_inputs(im_q, im_k, W_q, W_k, queues, label):
    f32 = np.float32
    im_q = np.asarray(im_q, dtype=f32)
    im_k = np.asarray(im_k, dtype=f32)
    W_q = np.asarray(W_q, dtype=f32)
    W_k = np.asarray(W_k, dtype=f32)
    queues = np.asarray(queues, dtype=f32)
    label = np.asarray(label)

    route = ((label.astype(np.int64) - 1) % G).astype(np.int64)
    order = np.argsort(route, kind="stable")
    counts = np.bincount(route, minlength=G)
    plan = _plan_from_counts(counts)

    im_q_s = im_q[order]
    im_k_s = im_k[order]

    imqt = _tile_cols(im_q_s, N)
    wqt = np.ascontiguousarray(
        W_q.reshape(DT, 128, C).transpose(1, 0, 2).reshape(128, DT * C))
    wkt = np.ascontiguousarray(
        W_k.reshape(DT, 128, C).transpose(1, 0, 2).reshape(128, DT * C))

    in_maps = []
    for i in range(NCORES):
        sl = slice(i * NLP, (i + 1) * NLP)
        in_maps.append({
            "imqt": imqt, "wqt": wqt, "wkt": wkt,
            "imqlp": _tile_cols(im_q_s[sl], NLP),
            "imklp": _tile_cols(im_k_s[sl], NLP),
            "qsh": np.ascontiguousarray(queues[:, :, i * KSH:(i + 1) * KSH]),
        })
    return plan, order, in_maps


def kernel(im_q, im_k, W_q, W_k, queues, label):
    from concourse.bass_utils import run_bass_kernel_spmd

    plan, order, in_maps = _stage_inputs(im_q, im_k, W_q, W_k, queues, label)
    nc = _get_program(plan)
    res = run_bass_kernel_spmd(nc, in_maps, core_ids=list(range(NCORES)))

    logits = np.empty((N, 1 + K), dtype=np.float32)
    lpos_sorted = np.concatenate(
        [res.results[i]["lpos"][0] for i in range(NCORES)])
    logits[order, 0] = lpos_sorted
    for i in range(NCORES):
        logits[order, 1 + i * KSH:1 + (i + 1) * KSH] = res.results[i]["lneg"]
    labels = np.zeros(N, dtype=np.int32)
    return logits, labels
